# revision 1
# baseline (speedup 1.0000x reference)
"""Trainium2 Bass kernel for nn_ASAMLayer (local-window sparse attention layer).

Sharding: token-parallel across 8 cores. 4096 tokens total -> 512 own tokens
per core, plus a 128-token halo on each side (within-batch, zero-padded at
batch edges) so the WINDOW=128 local attention needs no collectives.

On-chip layout: feature-major ("transposed") activations for all GEMMs.
LayerNorm gains/biases are folded into the following weight matrices on the
host (exact), softmax runs without max-subtraction (scores are bounded),
row-sums come free from the activation engine's accum_out, and the band mask
is a precomputed additive -1e6 tensor applied to scores in PSUM.
"""

import sys

import numpy as np

sys.path.insert(0, "/opt/trn_rl_repo")

import ml_dtypes  # noqa: E402

import concourse.bass as bass  # noqa: E402
from concourse import bacc  # noqa: E402
import concourse.mybir as mybir  # noqa: E402
import concourse.tile as tile  # noqa: E402
from concourse.bass_utils import run_bass_kernel_spmd  # noqa: E402

# Pin Exp and Ln to the joint set and Gelu to its anchor set so the act-table
# load pass emits few loads instead of thrashing per qtile. Set indices are
# preserved (unwanted sets are emptied, not removed).
import concourse.bacc as _bacc_mod  # noqa: E402
import concourse.hw_specs as _hw_specs  # noqa: E402

_orig_get_tables = _hw_specs.get_activation_tables


def _pinned_tables(module_arch):
    t = dict(_orig_get_tables(module_arch))
    keep = {"natural_log_exp_and_others", "gelu_and_others"}
    drop = {mybir.ActivationFunctionType.Exp,
            mybir.ActivationFunctionType.Ln,
            mybir.ActivationFunctionType.Gelu}
    return {name: (fns if name in keep else {f for f in fns if f not in drop})
            for name, fns in t.items()}


_bacc_mod.get_activation_tables = _pinned_tables


B, S, D = 2, 2048, 1024
H, DH = 16, 64
INNER = H * DH          # 1024
FF = 4 * D              # 4096
WINDOW = 128
EPS = 1e-5
SCALE = DH ** -0.5

NCORES = 8
OWN = (B * S) // NCORES          # 512 own tokens per core
HALO = WINDOW                    # 128
TLOC = OWN + 2 * HALO            # 768 local rows (halo'd)
P = 128
NQT = OWN // P                   # 4 query tiles
NKT = TLOC // P                  # 6 key tiles
KW = 3 * P                       # 384-wide key window per query tile

F32 = mybir.dt.float32
BF16 = mybir.dt.bfloat16
BF = ml_dtypes.bfloat16

MASK_NEG = -1.0e6


def _build_nc():
    nc = bacc.Bacc()

    x_s = nc.declare_dram_parameter("x_s", [TLOC, D], F32, isOutput=False)
    maskin = nc.declare_dram_parameter("maskin", [NQT, P, KW], F32, isOutput=False)
    wq_t = nc.declare_dram_parameter("wq_t", [8, P, 8, P], BF16, isOutput=False)
    wk_t = nc.declare_dram_parameter("wk_t", [8, P, 8, P], BF16, isOutput=False)
    wv_t = nc.declare_dram_parameter("wv_t", [P, 8, INNER], BF16, isOutput=False)
    wout_t = nc.declare_dram_parameter("wout_t", [8, P, 8, P], BF16, isOutput=False)
    wff1_t = nc.declare_dram_parameter("wff1_t", [32, P, 8, P], BF16, isOutput=False)
    wff2_t = nc.declare_dram_parameter("wff2_t", [8, P, 32, P], BF16, isOutput=False)
    bq_t = nc.declare_dram_parameter("bq_t", [P, 8], F32, isOutput=False)
    bk_t = nc.declare_dram_parameter("bk_t", [P, 8], F32, isOutput=False)
    bout_t = nc.declare_dram_parameter("bout_t", [P, 8], F32, isOutput=False)
    bff1_t = nc.declare_dram_parameter("bff1_t", [P, 32], F32, isOutput=False)
    bff2_t = nc.declare_dram_parameter("bff2_t", [P, 8], F32, isOutput=False)
    y = nc.declare_dram_parameter("y", [OWN, D], F32, isOutput=True)

    with tile.TileContext(nc) as tc:
        _emit(tc, nc, x_s, maskin, wq_t, wk_t, wv_t, wout_t, wff1_t, wff2_t,
              bq_t, bk_t, bout_t, bff1_t, bff2_t, y)
    nc.finalize()
    return nc


def _layernorm_tile(nc, pool, x_ap, out_bf16, eps_ap):
    """out_bf16 = (x - mean(x)) / sqrt(var(x) + EPS), bf16. x_ap [P, D] fp32."""
    xg = x_ap.rearrange("p (s f) -> p s f", f=512)
    stats = pool.tile([P, 2, 6], F32, tag="ln_stats")
    for s in range(2):
        nc.vector.bn_stats(out=stats[:, s, :], in_=xg[:, s, :])
    mv = pool.tile([P, 2], F32, tag="ln_mv")
    nc.vector.bn_aggr(out=mv[:], in_=stats[:])
    rstd = pool.tile([P, 1], F32, tag="ln_rstd")
    nc.scalar.activation(out=rstd[:], in_=mv[:, 1:2],
                         func=mybir.ActivationFunctionType.Ln,
                         bias=eps_ap, scale=1.0)
    nc.scalar.activation(out=rstd[:], in_=rstd[:],
                         func=mybir.ActivationFunctionType.Exp,
                         scale=-0.5)
    nc.vector.tensor_scalar(
        out=out_bf16[:], in0=x_ap,
        scalar1=mv[:, 0:1], scalar2=rstd[:],
        op0=mybir.AluOpType.subtract, op1=mybir.AluOpType.mult)


def _emit(tc, nc, x_s, maskin, wq_t, wk_t, wv_t, wout_t, wff1_t, wff2_t,
          bq_t, bk_t, bout_t, bff1_t, bff2_t, y):
    from contextlib import ExitStack
    ctx = ExitStack()
    Gelu = mybir.ActivationFunctionType.Gelu
    Exp = mybir.ActivationFunctionType.Exp
    Ident = mybir.ActivationFunctionType.Identity

    const = ctx.enter_context(tc.tile_pool(name="const", bufs=1))
    small = ctx.enter_context(tc.tile_pool(name="small", bufs=4))
    wst = ctx.enter_context(tc.tile_pool(name="wst", bufs=3))
    act = ctx.enter_context(tc.tile_pool(name="act", bufs=1))
    trans = ctx.enter_context(tc.tile_pool(name="trans", bufs=3))
    psum = ctx.enter_context(tc.tile_pool(name="psum", bufs=3, space="PSUM"))
    psum_kw = ctx.enter_context(tc.tile_pool(name="pskw", bufs=3, space="PSUM"))
    psum_small = ctx.enter_context(tc.tile_pool(name="psA", bufs=2, space="PSUM"))

    # ---- load wv (needed first), x, biases, masks ----
    wv = act.tile([P, 8, INNER], BF16, tag="big_wx2")   # 16KB/p (dies after V)
    nc.gpsimd.dma_start(out=wv[:], in_=wv_t[:])
    xt = act.tile([P, NKT, D], F32, tag="big_xh")       # 24KB/p
    for t in range(NKT):
        qeng = nc.sync if t < 3 else nc.scalar
        qeng.dma_start(out=xt[:, t, :], in_=x_s[t * P:(t + 1) * P, :])
    bq = const.tile([P, 8], F32, tag="bq")
    nc.gpsimd.dma_start(out=bq[:], in_=bq_t[:])
    bk = const.tile([P, 8], F32, tag="bk")
    nc.gpsimd.dma_start(out=bk[:], in_=bk_t[:])
    bout = const.tile([P, 8], F32, tag="bout")
    nc.gpsimd.dma_start(out=bout[:], in_=bout_t[:])
    bff1 = const.tile([P, 32], F32, tag="bff1")
    nc.gpsimd.dma_start(out=bff1[:], in_=bff1_t[:])
    bff2 = const.tile([P, 8], F32, tag="bff2")
    nc.gpsimd.dma_start(out=bff2[:], in_=bff2_t[:])
    eps_t = const.tile([P, 1], F32, tag="eps")
    nc.vector.memset(eps_t[:], EPS)
    maskt = act.tile([P, NQT, KW], F32, tag="mask")     # 6KB/p
    for q in range(NQT):
        nc.gpsimd.dma_start(out=maskt[:, q, :], in_=maskin[q])

    # ---- per token-tile: LN1 -> transpose -> V matmuls (PE starts early) ----
    yt = act.tile([P, 8, TLOC], BF16, tag="big_ya")     # 12KB/p
    vtok = act.tile([P, NKT, INNER], BF16, tag="vtok")  # 12KB/p
    for t in range(NKT):
        y16 = trans.tile([P, D], BF16, tag="y16t")
        _layernorm_tile(nc, small, xt[:, t, :], y16[:], eps_t[:])
        nc.sync.dma_start_transpose(yt[:, :, t * P:(t + 1) * P], y16[:])
        for half in range(2):
            ps = psum.tile([P, OWN], F32, tag="ps_big")
            sl = slice(half * 512, (half + 1) * 512)
            for k in range(8):
                nc.tensor.matmul(ps[:], yt[:, k, t * P:(t + 1) * P], wv[:, k, sl],
                                 start=(k == 0), stop=(k == 7))
            nc.vector.tensor_copy(out=vtok[:, t, sl], in_=ps[:])

    # ---- Q (own queries) and K (all local keys), feature-major ----
    qt_sb = act.tile([P, 8, OWN], BF16, tag="big_qoy")  # 8KB/p
    kt_sb = act.tile([P, 8, TLOC], BF16, tag="kt")      # 12KB/p
    for o in range(8):
        w = wst.tile([P, 8, P], BF16, tag="w_small")
        nc.sync.dma_start(out=w[:], in_=wq_t[o])
        ps = psum.tile([P, OWN], F32, tag="ps_big")
        for k in range(8):
            nc.tensor.matmul(ps[:], w[:, k, :], yt[:, k, HALO:HALO + OWN],
                             start=(k == 0), stop=(k == 7))
        nc.scalar.activation(out=qt_sb[:, o, :], in_=ps[:], func=Ident,
                             bias=bq[:, o:o + 1], scale=1.0)
    for o in range(8):
        w = wst.tile([P, 8, P], BF16, tag="w_small")
        nc.sync.dma_start(out=w[:], in_=wk_t[o])
        for half in range(2):
            ps = psum_kw.tile([P, KW], F32, tag="ps_kw")
            sl = slice(half * KW, (half + 1) * KW)
            for k in range(8):
                nc.tensor.matmul(ps[:], w[:, k, :], yt[:, k, sl],
                                 start=(k == 0), stop=(k == 7))
            nc.scalar.activation(out=kt_sb[:, o, sl], in_=ps[:], func=Ident,
                                 bias=bk[:, o:o + 1], scale=1.0)

    wout_sb = act.tile([P, 8, 8, P], BF16, tag="woutr")   # 16KB/p resident
    for o in range(8):
        nc.sync.dma_start(out=wout_sb[:, o], in_=wout_t[o])

    # ---- attention + per-qtile epilogue (out-proj, residual, LN2, Zt) ----
    avt = act.tile([P, 8, OWN], BF16, tag="big_ya")     # reuses yt slot
    x2 = act.tile([P, NQT, D], F32, tag="big_wx2")      # reuses wv slot
    z16 = act.tile([P, NQT, D], BF16, tag="big_zf")     # 8KB/p
    zt = act.tile([P, 8, OWN], BF16, tag="zt")          # 8KB/p
    for qtl in range(NQT):
        qsl = slice(qtl * P, (qtl + 1) * P)
        for i in range(8):
            for hh in range(2):
                h = 2 * i + hh
                hs = slice(hh * DH, (hh + 1) * DH)
                ksl = slice(qtl * P, qtl * P + KW)
                sc = psum_kw.tile([P, KW], F32, tag="ps_kw")
                nc.tensor.matmul(sc[:], qt_sb[hs, i, qsl], kt_sb[hs, i, ksl],
                                 start=True, stop=True)
                scm = trans.tile([P, KW], F32, tag="scm")
                nc.vector.tensor_tensor(out=scm[:], in0=sc[:],
                                        in1=maskt[:, qtl, :],
                                        op=mybir.AluOpType.add)
                ptm = trans.tile([P, KW], BF16, tag="ptm")
                ssum = small.tile([P, 1], F32, tag="ssum")
                nc.scalar.activation(out=ptm[:], in_=scm[:], func=Exp,
                                     scale=SCALE, accum_out=ssum[:])
                nc.vector.reciprocal(out=ssum[:], in_=ssum[:])
                nc.vector.tensor_scalar_mul(ptm[:], ptm[:], ssum[:])
                ptt = trans.tile([P, 3, P], BF16, tag="ptt")
                for e in range(3):
                    nc.sync.dma_start_transpose(ptt[:, e, :],
                                                ptm[:, e * P:(e + 1) * P])
                av = psum_small.tile([DH, P], F32, tag="ps_av")
                for e in range(3):
                    nc.tensor.matmul(av[:], vtok[:, qtl + e, h * DH:(h + 1) * DH],
                                     ptt[:, e, :], start=(e == 0), stop=(e == 2))
                nc.any.tensor_copy(out=avt[hs, i, qsl], in_=av[:])
        # out-projection for this qtile, then residual + LN2 + Zt
        osb_q = trans.tile([P, 8, P], BF16, tag="osbq")
        otok_q = trans.tile([P, D], BF16, tag="otokq")
        for o in range(8):
            ps = psum_kw.tile([P, KW], F32, tag="ps_kw")
            for k in range(8):
                nc.tensor.matmul(ps[:, :P], wout_sb[:, o, k, :], avt[:, k, qsl],
                                 start=(k == 0), stop=(k == 7))
            nc.scalar.activation(out=osb_q[:, o, :], in_=ps[:, :P], func=Ident,
                                 bias=bout[:, o:o + 1], scale=1.0)
            nc.sync.dma_start_transpose(otok_q[:, o * P:(o + 1) * P],
                                        osb_q[:, o, :])
            osl = slice(o * P, (o + 1) * P)
            nc.vector.tensor_tensor(out=x2[:, qtl, osl],
                                    in0=xt[:, qtl + 1, osl],
                                    in1=otok_q[:, osl],
                                    op=mybir.AluOpType.add)
        _layernorm_tile(nc, small, x2[:, qtl, :], z16[:, qtl, :], eps_t[:])
        nc.sync.dma_start_transpose(zt[:, :, qtl * P:(qtl + 1) * P],
                                    z16[:, qtl, :])

    # ---- FFN ----
    ht = act.tile([P, 32, OWN], BF16, tag="big_xh")     # reuses x slot
    for o in range(32):
        w = wst.tile([P, 8, P], BF16, tag="w_small")
        nc.scalar.dma_start(out=w[:], in_=wff1_t[o])
        ps = psum.tile([P, OWN], F32, tag="ps_big")
        for q in range(NQT):
            for k in range(8):
                nc.tensor.matmul(ps[:, q * P:(q + 1) * P], w[:, k, :],
                                 zt[:, k, q * P:(q + 1) * P],
                                 start=(k == 0), stop=(k == 7))
        nc.scalar.activation(out=ht[:, o, :], in_=ps[:], func=Gelu,
                             bias=bff1[:, o:o + 1], scale=1.0)
    fsb = act.tile([P, 8, OWN], BF16, tag="fsb")        # 8KB/p
    ftok = act.tile([P, NQT, D], BF16, tag="big_zf")    # reuses z16 slot
    yout = act.tile([P, NQT, D], F32, tag="big_qoy")    # reuses qt slot
    for o in range(8):
        w = wst.tile([P, 32, P], BF16, tag="w_ff2")
        nc.sync.dma_start(out=w[:], in_=wff2_t[o])
        ps = psum.tile([P, OWN], F32, tag="ps_big")
        for k in range(32):
            nc.tensor.matmul(ps[:], w[:, k, :], ht[:, k, :],
                             start=(k == 0), stop=(k == 31))
        nc.scalar.activation(out=fsb[:, o, :], in_=ps[:], func=Ident,
                             bias=bff2[:, o:o + 1], scale=1.0)
        nc.sync.dma_start_transpose(ftok[:, :, o * P:(o + 1) * P], fsb[:, o, :])
        osl = slice(o * P, (o + 1) * P)
        for q in range(NQT):
            nc.vector.tensor_tensor(out=yout[:, q, osl], in0=x2[:, q, osl],
                                    in1=ftok[:, q, osl],
                                    op=mybir.AluOpType.add)
    for q in range(NQT):
        nc.sync.dma_start(out=y[q * P:(q + 1) * P, :], in_=yout[:, q, :])
    ctx.close()


def _host_prep(x, ln1_g, ln1_b, w_qkv, w_out, b_out, ln2_g, ln2_b,
               w_ff1, b_ff1, w_ff2, b_ff2):
    """Fold LN affine params into weights, pre-transpose/tile, build per-core
    input maps."""
    f8 = np.float64
    wqkv_eff = (w_qkv.astype(f8) * ln1_g.astype(f8)[None, :])
    bqkv_eff = w_qkv.astype(f8) @ ln1_b.astype(f8)
    wq, wk, wv = wqkv_eff[:INNER], wqkv_eff[INNER:2 * INNER], wqkv_eff[2 * INNER:]
    bq_v, bk_v, bv_v = bqkv_eff[:INNER], bqkv_eff[INNER:2 * INNER], bqkv_eff[2 * INNER:]
    bout_eff = b_out.astype(f8) + w_out.astype(f8) @ bv_v
    wff1_eff = w_ff1.astype(f8) * ln2_g.astype(f8)[None, :]
    bff1_eff = b_ff1.astype(f8) + w_ff1.astype(f8) @ ln2_b.astype(f8)

    def lhst(w, ko, no):  # w [K, N] -> [no, 128, ko, 128] bf16 (p = K within tile)
        a = np.ascontiguousarray(
            w.reshape(ko, P, no, P).transpose(2, 1, 0, 3)).astype(BF)
        return a

    wq_t = lhst(wq.T, 8, 8)
    wk_t = lhst(wk.T, 8, 8)
    wv_t = np.ascontiguousarray(wv.T.reshape(8, P, INNER).transpose(1, 0, 2)).astype(BF)
    wout_t = lhst(w_out.astype(f8).T, 8, 8)
    wff1_t = lhst(wff1_eff.T, 8, 32)
    wff2_t = lhst(w_ff2.astype(f8).T, 32, 8)

    def colmaj(b, n):  # [n*128] -> [128, n] fp32
        return np.ascontiguousarray(b.reshape(n, P).T).astype(np.float32)

    bq_t = colmaj(bq_v, 8)
    bk_t = colmaj(bk_v, 8)
    bout_t = colmaj(bout_eff, 8)
    bff1_t = colmaj(bff1_eff, 32)
    bff2_t = colmaj(b_ff2.astype(f8), 8)

    xf = x.reshape(B * S, D).astype(np.float32)
    in_maps = []
    for c in range(NCORES):
        b = c // (NCORES // B)
        s0 = (c % (NCORES // B)) * OWN          # within-batch start of own rows
        lo, hi = s0 - HALO, s0 + OWN + HALO
        xs = np.zeros((TLOC, D), np.float32)
        clo, chi = max(lo, 0), min(hi, S)
        xs[clo - lo:chi - lo] = xf[b * S + clo:b * S + chi]
        # additive mask [NQT, 128, 384]: query r in tile qtl, key col ccol
        q_idx = s0 + np.arange(OWN)             # within-batch query positions
        mask = np.zeros((NQT, P, KW), np.float32)
        for qtl in range(NQT):
            qq = q_idx[qtl * P:(qtl + 1) * P][:, None]      # [128,1]
            kk = (s0 + qtl * P - HALO) + np.arange(KW)[None, :]
            bad = (np.abs(kk - qq) > WINDOW) | (kk < 0) | (kk >= S)
            mask[qtl][bad] = MASK_NEG
        in_maps.append(dict(
            x_s=xs, maskin=mask, wq_t=wq_t, wk_t=wk_t, wv_t=wv_t,
            wout_t=wout_t, wff1_t=wff1_t, wff2_t=wff2_t,
            bq_t=bq_t, bk_t=bk_t, bout_t=bout_t, bff1_t=bff1_t, bff2_t=bff2_t))
    return in_maps


_NC_CACHE = {}


def kernel(x, ln1_g, ln1_b, w_qkv, w_out, b_out, ln2_g, ln2_b,
           w_ff1, b_ff1, w_ff2, b_ff2, _trace=False):
    in_maps = _host_prep(x, ln1_g, ln1_b, w_qkv, w_out, b_out,
                         ln2_g, ln2_b, w_ff1, b_ff1, w_ff2, b_ff2)
    if "nc" not in _NC_CACHE:
        _NC_CACHE["nc"] = _build_nc()
    nc = _NC_CACHE["nc"]
    res = run_bass_kernel_spmd(nc, in_maps, core_ids=list(range(NCORES)),
                               trace=_trace)
    if _trace:
        _NC_CACHE["last"] = res
    out = np.concatenate([res.results[c]["y"] for c in range(NCORES)], axis=0)
    return out.reshape(B, S, D).astype(np.float32)



# revision 2
# speedup vs baseline: 2.0767x; 2.0767x over previous
"""Trainium2 Bass kernel for nn_ASAMLayer (local-window sparse attention layer).

v2: fp8 DoubleRow GEMMs + transposed-score attention.

Sharding: token-parallel across 8 cores (512 own tokens + 128-halo each side).

Key structure per core:
- LN1 on token-major f32 x -> bf16 -> DmaTranspose -> fp8 feature-major x_hat.
- V/Q/K/out-proj/FFN1/FFN2 as fp8e4m3 DoubleRow matmuls (2 contraction
  chunks per instruction at 0.5 cycles/row).  Weights are pre-scaled on the
  host (x8/x32/x64) to escape fp8's denormal range; the scales cancel in
  the exp scale, the gelu input scale, a final 1/64 output scale, and the
  softmax-denominator "validity" column.
- Attention computed transposed: scores land as [key, query] tiles so the
  softmax probabilities feed the AV matmul directly as the stationary
  operand (no per-head transposes).  The band mask is a multiplicative
  bf16 tensor applied after exp; sequence-edge validity is handled by a
  scaled indicator column appended to V, which simultaneously produces the
  softmax denominators inside the AV matmul (renorm is deferred to the AV
  evacuation, where 1/sum is a per-partition scalar).
- Residual adds in f32 token-major; biases ride evacuations (per-partition
  APs) or rank-1 ones-row matmuls for token-major outputs.
"""

import sys

import numpy as np

sys.path.insert(0, "/opt/trn_rl_repo")

import ml_dtypes  # noqa: E402

import concourse.bass as bass  # noqa: E402
from concourse import bacc  # noqa: E402
import concourse.mybir as mybir  # noqa: E402
import concourse.tile as tile  # noqa: E402
from concourse.bass_utils import run_bass_kernel_spmd  # noqa: E402

# Pin Exp and Ln to the joint set and Gelu to its anchor set so the act-table
# load pass emits few loads instead of thrashing. Set indices are preserved.
import concourse.bacc as _bacc_mod  # noqa: E402
import concourse.hw_specs as _hw_specs  # noqa: E402

_orig_get_tables = _hw_specs.get_activation_tables


def _pinned_tables(module_arch):
    t = dict(_orig_get_tables(module_arch))
    keep = {"natural_log_exp_and_others", "gelu_and_others"}
    drop = {mybir.ActivationFunctionType.Exp,
            mybir.ActivationFunctionType.Ln,
            mybir.ActivationFunctionType.Gelu}
    return {name: (fns if name in keep else {f for f in fns if f not in drop})
            for name, fns in t.items()}


_bacc_mod.get_activation_tables = _pinned_tables


B, S, D = 2, 2048, 1024
H, DH = 16, 64
INNER = H * DH          # 1024
FF = 4 * D              # 4096
WINDOW = 128
EPS = 1e-5
SCALE = DH ** -0.5

NCORES = 8
OWN = (B * S) // NCORES          # 512 own tokens per core
HALO = WINDOW                    # 128
TLOC = OWN + 2 * HALO            # 768 local rows (halo'd)
P = 128
NQT = OWN // P                   # 4 query tiles
NKT = TLOC // P                  # 6 key tiles

F32 = mybir.dt.float32
BF16 = mybir.dt.bfloat16
FP8 = mybir.dt.float8e4
BF = ml_dtypes.bfloat16
F8NP = ml_dtypes.float8_e4m3
DR = mybir.MatmulPerfMode.DoubleRow

# fp8 weight pre-scales (cancelled on-device; see module docstring)
SQ = 8.0     # wq, wk (and their biases)
SV = 8.0     # wv
SO = 8.0     # wout
S1 = 32.0    # wff1
S2 = 64.0    # wff2
VAL = SV * SO               # validity-column value (64)
ESC = SCALE / (SQ * SQ)     # exp scale (1/512)
HV = 65                     # per-head V stride (64 dims + validity col)


def _build_nc():
    nc = bacc.Bacc()

    x_s = nc.declare_dram_parameter("x_s", [TLOC, D], F32, isOutput=False)
    wq8 = nc.declare_dram_parameter("wq8", [P, 8, INNER], FP8, isOutput=False)
    wk8 = nc.declare_dram_parameter("wk8", [P, 8, INNER], FP8, isOutput=False)
    wv8 = nc.declare_dram_parameter("wv8", [P, 8, INNER], FP8, isOutput=False)
    wo8 = nc.declare_dram_parameter("wo8", [P, 8, D], FP8, isOutput=False)
    w18 = nc.declare_dram_parameter("w18", [4, P, 8, 1024], FP8, isOutput=False)
    w28 = nc.declare_dram_parameter("w28", [4, P, 32, 256], FP8, isOutput=False)
    bq8 = nc.declare_dram_parameter("bq8", [P, 8], F32, isOutput=False)
    bk8 = nc.declare_dram_parameter("bk8", [P, 8], F32, isOutput=False)
    bf1 = nc.declare_dram_parameter("bf1", [P, 32], F32, isOutput=False)
    brow = nc.declare_dram_parameter("brow", [1, 2 * D], BF16, isOutput=False)
    bandm = nc.declare_dram_parameter("bandm", [P, 2, 384], BF16, isOutput=False)
    vald = nc.declare_dram_parameter("vald", [P, NKT * H], BF16, isOutput=False)
    y = nc.declare_dram_parameter("y", [OWN, D], F32, isOutput=True)

    with tile.TileContext(nc) as tc:
        _emit(tc, nc, x_s, wq8, wk8, wv8, wo8, w18, w28,
              bq8, bk8, bf1, brow, bandm, vald, y)
    nc.finalize()
    return nc


def _ln_stats(nc, small, x_ap, eps_ap):
    """Returns (mv, rstd): mv[:,0:1]=mean, rstd=[P,1] 1/sqrt(var+EPS)."""
    Ln = mybir.ActivationFunctionType.Ln
    Exp = mybir.ActivationFunctionType.Exp
    xg = x_ap.rearrange("p (s f) -> p s f", f=512)
    stats = small.tile([P, 2, 6], F32, tag="ln_stats")
    for s in range(2):
        nc.vector.bn_stats(out=stats[:, s, :], in_=xg[:, s, :])
    mv = small.tile([P, 2], F32, tag="ln_mv")
    nc.vector.bn_aggr(out=mv[:], in_=stats[:])
    rstd = small.tile([P, 1], F32, tag="ln_rstd")
    nc.scalar.activation(out=rstd[:], in_=mv[:, 1:2], func=Ln,
                         bias=eps_ap, scale=1.0)
    nc.scalar.activation(out=rstd[:], in_=rstd[:], func=Exp, scale=-0.5)
    return mv, rstd


def _emit(tc, nc, x_s, wq8, wk8, wv8, wo8, w18, w28,
          bq8, bk8, bf1, brow, bandm, vald, y):
    from contextlib import ExitStack
    ctx = ExitStack()
    Gelu = mybir.ActivationFunctionType.Gelu
    Exp = mybir.ActivationFunctionType.Exp
    Ident = mybir.ActivationFunctionType.Identity
    Copy = mybir.ActivationFunctionType.Copy
    ADD = mybir.AluOpType.add
    MUL = mybir.AluOpType.mult
    SUB = mybir.AluOpType.subtract

    const = ctx.enter_context(tc.tile_pool(name="const", bufs=1))
    small = ctx.enter_context(tc.tile_pool(name="small", bufs=4))
    big = ctx.enter_context(tc.tile_pool(name="big", bufs=1))
    wst = ctx.enter_context(tc.tile_pool(name="wst", bufs=2))
    trans = ctx.enter_context(tc.tile_pool(name="trans", bufs=2))
    hot = ctx.enter_context(tc.tile_pool(name="hot", bufs=3))
    ps_big = ctx.enter_context(tc.tile_pool(name="psbig", bufs=2, space="PSUM"))
    ps_st = ctx.enter_context(tc.tile_pool(name="psst", bufs=2, space="PSUM"))
    ps_av = ctx.enter_context(tc.tile_pool(name="psav", bufs=2, space="PSUM"))

    # ---- constants ----
    bq_t = const.tile([P, 8], F32, tag="bq")
    nc.gpsimd.dma_start(out=bq_t[:], in_=bq8[:])
    bk_t = const.tile([P, 8], F32, tag="bk")
    nc.gpsimd.dma_start(out=bk_t[:], in_=bk8[:])
    bf1_t = const.tile([P, 32], F32, tag="bf1")
    nc.gpsimd.dma_start(out=bf1_t[:], in_=bf1[:])
    brow_t = const.tile([1, 2 * D], BF16, tag="brow")
    nc.gpsimd.dma_start(out=brow_t[:], in_=brow[:])
    band_t = const.tile([P, 2, 384], BF16, tag="band")
    nc.gpsimd.dma_start(out=band_t[:], in_=bandm[:])
    ones_t = const.tile([1, P], BF16, tag="ones")
    nc.vector.memset(ones_t[:], 1.0)
    eps_t = const.tile([P, 1], F32, tag="eps")
    nc.vector.memset(eps_t[:], EPS)

    # ---- input x ----
    xt = big.tile([P, NKT, D], F32, tag="xt")          # 24KB/p
    for t in range(NKT):
        nc.sync.dma_start(out=xt[:, t, :], in_=x_s[t * P:(t + 1) * P, :])

    # ---- weights (big fp8 DMAs via SWDGE on Pool) ----
    wv_t = big.tile([P, 8, INNER], FP8, tag="wv8")     # 8KB/p
    nc.gpsimd.dma_start(out=wv_t[:], in_=wv8[:])
    wq_t = big.tile([P, 8, INNER], FP8, tag="wq8")
    nc.gpsimd.dma_start(out=wq_t[:], in_=wq8[:])
    wk_t = big.tile([P, 8, INNER], FP8, tag="wk8")
    nc.gpsimd.dma_start(out=wk_t[:], in_=wk8[:])
    wo_t = big.tile([P, 8, D], FP8, tag="wo8")
    nc.gpsimd.dma_start(out=wo_t[:], in_=wo8[:])

    # ---- LN1 -> bf16 token-major -> transpose -> fp8 feature-major ----
    yt8 = big.tile([P, 8, TLOC], FP8, tag="yt8")       # 6KB/p
    for t in range(NKT):
        mv, rstd = _ln_stats(nc, small, xt[:, t, :], eps_t[:])
        y16 = trans.tile([P, D], BF16, tag="ln16")
        nc.vector.tensor_scalar(out=y16[:], in0=xt[:, t, :],
                                scalar1=mv[:, 0:1], scalar2=rstd[:],
                                op0=SUB, op1=MUL)
        ytt = trans.tile([P, 8, P], BF16, tag="tp16")
        nc.sync.dma_start_transpose(ytt[:], y16[:])
        nc.scalar.activation(out=yt8[:, :, t * P:(t + 1) * P], in_=ytt[:],
                             func=Copy, scale=1.0)

    # ---- V GEMM (fp8 DR): out [tok,512] per (t, half) ----
    vtok = big.tile([P, NKT, H * HV], BF16, tag="vtok")  # ~12.2KB/p
    for t in range(NKT):
        for half in range(2):
            ps = ps_big.tile([P, 512], F32, tag="ps_big")
            for j in range(4):
                nc.tensor.matmul(ps[:], yt8[:, 2 * j:2 * j + 2, t * P:(t + 1) * P],
                                 wv_t[:, 2 * j:2 * j + 2, half * 512:(half + 1) * 512],
                                 start=(j == 0), stop=(j == 3), perf_mode=DR)
            ov = vtok[:, t, half * 8 * HV:(half + 1) * 8 * HV]
            ov = ov.rearrange("p (h d) -> p h d", d=HV)[:, :, 0:64]
            nc.scalar.activation(out=ov, in_=ps[:].rearrange(
                "p (h d) -> p h d", d=64), func=Copy, scale=1.0)
    # validity indicator column (also the softmax-denominator weights)
    vapd = vtok[:].rearrange("p t (h d) -> p t h d", d=HV)[:, :, :, 64]
    nc.sync.dma_start(out=vapd, in_=vald[:].rearrange(
        "p (t h) -> p t h", h=H))

    # ---- Q GEMM: out [ofeat 128, own 512] per o-tile ----
    qt16 = big.tile([P, 8, OWN], BF16, tag="qt16")     # 8KB/p
    for o in range(8):
        ps = ps_big.tile([P, 512], F32, tag="ps_big")
        for j in range(4):
            nc.tensor.matmul(ps[:], wq_t[:, 2 * j:2 * j + 2, o * P:(o + 1) * P],
                             yt8[:, 2 * j:2 * j + 2, HALO:HALO + OWN],
                             start=(j == 0), stop=(j == 3), perf_mode=DR)
        nc.scalar.activation(out=qt16[:, o, :], in_=ps[:], func=Ident,
                             bias=bq_t[:, o:o + 1], scale=1.0)

    # ---- K GEMM: out [ofeat 128, 384] per (o, half) over all 768 ----
    kt16 = big.tile([P, 8, TLOC], BF16, tag="kt16")    # 12KB/p
    for o in range(8):
        for half in range(2):
            ps = ps_big.tile([P, 512], F32, tag="ps_big")
            for j in range(4):
                nc.tensor.matmul(ps[:, 0:384],
                                 wk_t[:, 2 * j:2 * j + 2, o * P:(o + 1) * P],
                                 yt8[:, 2 * j:2 * j + 2, half * 384:(half + 1) * 384],
                                 start=(j == 0), stop=(j == 3), perf_mode=DR)
            nc.scalar.activation(out=kt16[:, o, half * 384:(half + 1) * 384],
                                 in_=ps[:, 0:384], func=Ident,
                                 bias=bk_t[:, o:o + 1], scale=1.0)

    # ---- attention (transposed scores) + out-proj + LN2, per qtl ----
    x2 = big.tile([P, NQT, D], F32, tag="x2")          # 16KB/p
    zt8 = big.tile([P, 8, OWN], FP8, tag="zt8")        # 4KB/p
    for qtl in range(NQT):
        att = trans.tile([P, INNER], BF16, tag="att_t")
        for n in range(4):              # 4-head groups
            av = ps_av.tile([P, 4, HV], F32, tag="ps_av")
            for mm in range(2):         # 2-head subgroups
                m = 2 * n + mm
                st = ps_st.tile([P, 2, 512], F32, tag="ps_st")
                for g in range(2):
                    hs = slice(64 * g, 64 * g + 64)
                    for e in range(3):
                        nc.tensor.matmul(
                            st[:, g, e * P:(e + 1) * P],
                            kt16[hs, m, (qtl + e) * P:(qtl + e + 1) * P],
                            qt16[hs, m, qtl * P:(qtl + 1) * P],
                            start=True, stop=True)
                pt = hot.tile([P, 2, 384], BF16, tag="pt16")
                nc.scalar.activation(out=pt[:], in_=st[:, :, 0:384],
                                     func=Exp, scale=ESC)
                ptm = hot.tile([P, 2, 3, P], BF16, tag="ptm16")
                nc.vector.tensor_tensor(
                    out=ptm[:], in0=pt[:].rearrange("p g (e q) -> p g e q", q=P),
                    in1=band_t[:].rearrange("p g (e q) -> p g e q", q=P),
                    op=MUL)
                for g in range(2):
                    h = 2 * m + g
                    gg = 2 * mm + g
                    for e in range(3):
                        nc.tensor.matmul(
                            av[:, gg, :], ptm[:, g, e, :],
                            vtok[:, qtl + e, h * HV:(h + 1) * HV],
                            start=(e == 0), stop=(e == 2))
            rc = small.tile([P, 4], F32, tag="rc4")
            nc.vector.reciprocal(out=rc[:], in_=av[:, :, 64])
            oatt = att[:, n * 256:(n + 1) * 256].rearrange(
                "p (g d) -> p g d", d=64)
            nc.vector.tensor_tensor(out=oatt, in0=av[:, :, 0:64],
                                    in1=rc[:].broadcast_to([P, 4, 64]), op=MUL)
        atf16 = trans.tile([P, 8, P], BF16, tag="tp16")
        nc.sync.dma_start_transpose(atf16[:], att[:])
        atf8 = trans.tile([P, 8, P], FP8, tag="atf8")
        nc.vector.tensor_copy(out=atf8[:], in_=atf16[:])
        # out-projection + bias + residual
        for half in range(2):
            ps = ps_big.tile([P, 512], F32, tag="ps_big")
            for j in range(4):
                nc.tensor.matmul(ps[:], atf8[:, 2 * j:2 * j + 2, :],
                                 wo_t[:, 2 * j:2 * j + 2, half * 512:(half + 1) * 512],
                                 start=(j == 0), stop=False, perf_mode=DR)
            nc.tensor.matmul(ps[:], ones_t[:],
                             brow_t[:, half * 512:(half + 1) * 512],
                             start=False, stop=True)
            nc.vector.tensor_tensor(
                out=x2[:, qtl, half * 512:(half + 1) * 512], in0=ps[:],
                in1=xt[:, qtl + 1, half * 512:(half + 1) * 512], op=ADD)
        # LN2 -> bf16 -> transpose -> fp8
        mv, rstd = _ln_stats(nc, small, x2[:, qtl, :], eps_t[:])
        z16 = trans.tile([P, D], BF16, tag="ln16")
        nc.vector.tensor_scalar(out=z16[:], in0=x2[:, qtl, :],
                                scalar1=mv[:, 0:1], scalar2=rstd[:],
                                op0=SUB, op1=MUL)
        ztt = trans.tile([P, 8, P], BF16, tag="tp16")
        nc.sync.dma_start_transpose(ztt[:], z16[:])
        nc.scalar.activation(out=zt8[:, :, qtl * P:(qtl + 1) * P], in_=ztt[:],
                             func=Copy, scale=1.0)

    # ---- FFN1 (fp8 DR), weights streamed in 4 groups of 8 o-tiles ----
    h18 = big.tile([P, 32, OWN], FP8, tag="h18")       # 16KB/p
    for g in range(4):
        w1g = wst.tile([P, 8, 1024], FP8, tag="w1g")
        nc.gpsimd.dma_start(out=w1g[:], in_=w18[g])
        for ol in range(8):
            o = 8 * g + ol
            ps = ps_big.tile([P, 512], F32, tag="ps_big")
            for j in range(4):
                nc.tensor.matmul(ps[:], w1g[:, 2 * j:2 * j + 2, ol * P:(ol + 1) * P],
                                 zt8[:, 2 * j:2 * j + 2, :],
                                 start=(j == 0), stop=(j == 3), perf_mode=DR)
            nc.scalar.activation(out=h18[:, o, :], in_=ps[:], func=Gelu,
                                 bias=bf1_t[:, o:o + 1], scale=1.0 / S1)

    # ---- FFN2 (fp8 DR), weights streamed in 4 output-quarters ----
    yo = big.tile([P, NKT, D], F32, tag="xt")          # reuses xt slot
    for qq in range(4):
        w2q = wst.tile([P, 32, 256], FP8, tag="w2q")
        nc.gpsimd.dma_start(out=w2q[:], in_=w28[qq])
        for qtl in range(NQT):
            ps = ps_big.tile([P, 512], F32, tag="ps_big")
            for j in range(16):
                nc.tensor.matmul(ps[:, 0:256],
                                 h18[:, 2 * j:2 * j + 2, qtl * P:(qtl + 1) * P],
                                 w2q[:, 2 * j:2 * j + 2, :],
                                 start=(j == 0), stop=False, perf_mode=DR)
            nc.tensor.matmul(ps[:, 0:256], ones_t[:],
                             brow_t[:, D + qq * 256:D + (qq + 1) * 256],
                             start=False, stop=True)
            f2t = trans.tile([P, 256], BF16, tag="f2t")
            nc.scalar.activation(out=f2t[:], in_=ps[:, 0:256], func=Copy,
                                 scale=1.0 / S2)
            nc.vector.tensor_tensor(
                out=yo[:, qtl, qq * 256:(qq + 1) * 256], in0=f2t[:],
                in1=x2[:, qtl, qq * 256:(qq + 1) * 256], op=ADD)
    for qtl in range(NQT):
        nc.sync.dma_start(out=y[qtl * P:(qtl + 1) * P, :], in_=yo[:, qtl, :])
    ctx.close()


def _host_prep(x, ln1_g, ln1_b, w_qkv, w_out, b_out, ln2_g, ln2_b,
               w_ff1, b_ff1, w_ff2, b_ff2):
    """Fold LN affines into weights, scale + fp8-cast, build per-core maps."""
    f8 = np.float64
    wqkv_eff = (w_qkv.astype(f8) * ln1_g.astype(f8)[None, :])
    bqkv_eff = w_qkv.astype(f8) @ ln1_b.astype(f8)
    wq, wk, wv = wqkv_eff[:INNER], wqkv_eff[INNER:2 * INNER], wqkv_eff[2 * INNER:]
    bq_v, bk_v, bv_v = (bqkv_eff[:INNER], bqkv_eff[INNER:2 * INNER],
                        bqkv_eff[2 * INNER:])
    bout_eff = b_out.astype(f8) + w_out.astype(f8) @ bv_v
    wff1_eff = w_ff1.astype(f8) * ln2_g.astype(f8)[None, :]
    bff1_eff = b_ff1.astype(f8) + w_ff1.astype(f8) @ ln2_b.astype(f8)

    def fm8(w, scale):
        # w [N, Dk] -> [128, Dk//128, N] fp8: [p, kc, n] = w[n, 128*kc+p]*scale
        dk = w.shape[1]
        a = (w.T * scale).reshape(dk // P, P, -1).transpose(1, 0, 2)
        return np.ascontiguousarray(a).astype(F8NP)

    wq8 = fm8(wq, SQ)
    wk8 = fm8(wk, SQ)
    wv8 = fm8(wv, SV)
    wo8 = fm8(w_out.astype(f8), SO)
    w18_full = fm8(wff1_eff, S1)                     # [128, 8, 4096]
    w18 = np.ascontiguousarray(
        w18_full.reshape(P, 8, 4, 1024).transpose(2, 0, 1, 3))
    w28_full = fm8(w_ff2.astype(f8), S2)             # [128, 32, 1024]
    w28 = np.ascontiguousarray(
        w28_full.reshape(P, 32, 4, 256).transpose(2, 0, 1, 3))

    def colmaj(b, n, scale=1.0):
        return np.ascontiguousarray(
            (b * scale).reshape(n, P).T).astype(np.float32)

    bq8 = colmaj(bq_v, 8, SQ)
    bk8 = colmaj(bk_v, 8, SQ)
    bf1 = colmaj(bff1_eff, 32)
    brow = np.concatenate([bout_eff, b_ff2.astype(f8) * S2]).reshape(1, 2 * D)
    brow = brow.astype(BF)

    # band mask [p, g, 128e+r]: allowed iff 0 <= (128e+p) - r <= 256
    pp = np.arange(P)
    ee = np.arange(3)
    rr = np.arange(P)
    kk = (128 * ee[:, None] + pp[None, :])           # [e, p]
    dd = kk[:, :, None] - rr[None, None, :]          # [e, p, r]
    band = ((dd >= 0) & (dd <= 2 * WINDOW)).astype(BF)  # [e, p, r]
    bandm = np.ascontiguousarray(
        np.broadcast_to(band.transpose(1, 0, 2)[:, None], (P, 2, 3, P))
    ).reshape(P, 2, 384)

    xf = x.reshape(B * S, D).astype(np.float32)
    in_maps = []
    for c in range(NCORES):
        b = c // (NCORES // B)
        s0 = (c % (NCORES // B)) * OWN
        lo, hi = s0 - HALO, s0 + OWN + HALO
        xs = np.zeros((TLOC, D), np.float32)
        clo, chi = max(lo, 0), min(hi, S)
        xs[clo - lo:chi - lo] = xf[b * S + clo:b * S + chi]
        lt = np.arange(TLOC)
        valid = ((s0 - HALO + lt) >= 0) & ((s0 - HALO + lt) < S)
        vald = np.where(valid[:, None], np.float32(VAL), np.float32(0.0))
        vald = np.broadcast_to(vald, (TLOC, H)).reshape(NKT, P, H)
        vald = np.ascontiguousarray(vald.transpose(1, 0, 2)).reshape(
            P, NKT * H).astype(BF)
        in_maps.append(dict(
            x_s=xs, wq8=wq8, wk8=wk8, wv8=wv8, wo8=wo8, w18=w18, w28=w28,
            bq8=bq8, bk8=bk8, bf1=bf1, brow=brow, bandm=bandm, vald=vald))
    return in_maps


_NC_CACHE = {}


def kernel(x, ln1_g, ln1_b, w_qkv, w_out, b_out, ln2_g, ln2_b,
           w_ff1, b_ff1, w_ff2, b_ff2, _trace=False):
    in_maps = _host_prep(x, ln1_g, ln1_b, w_qkv, w_out, b_out,
                         ln2_g, ln2_b, w_ff1, b_ff1, w_ff2, b_ff2)
    if "nc" not in _NC_CACHE:
        _NC_CACHE["nc"] = _build_nc()
    nc = _NC_CACHE["nc"]
    res = run_bass_kernel_spmd(nc, in_maps, core_ids=list(range(NCORES)),
                               trace=_trace)
    if _trace:
        _NC_CACHE["last"] = res
    out = np.concatenate([res.results[c]["y"] for c in range(NCORES)], axis=0)
    return out.reshape(B, S, D).astype(np.float32)


# revision 3
# speedup vs baseline: 2.1340x; 1.0276x over previous
"""Trainium2 Bass kernel for nn_ASAMLayer (local-window sparse attention layer).

v3: fp8 DoubleRow everywhere (incl. scores) + matmul-injected band mask.

Sharding: token-parallel across 8 cores (512 own tokens + 128-halo each side).

Structure per core:
- LN1 on token-major f32 x -> bf16 -> DmaTranspose -> fp8 feature-major x_hat.
- All big GEMMs (V/Q/K/scores/out-proj/FFN1/FFN2) are fp8e4m3 DoubleRow
  matmuls (2 contraction chunks per instruction at 0.5 cycles/row).  Weights
  are pre-scaled on the host (x8/x32/x64) to escape fp8's denormal range;
  the scales cancel in the exp scale, the gelu input scale, a fused 1/64
  output scale, and the softmax-denominator validity column.
- Attention computed transposed: scores land as [key, query] tiles so the
  softmax probabilities feed the AV matmul directly as the stationary
  operand (no per-head transposes).  The |i-j|<=128 band mask decomposes
  per 128-key chunk into [lower-tri, all-ones, upper-tri]; the triangle
  chunks get a -1e6 additive mask injected into PSUM by a bf16 matmul
  (triM^T @ I) before the exp, so no vector-engine mask pass exists.
  Sequence-edge validity is a scaled indicator column appended to V which
  simultaneously produces the softmax denominators inside the AV matmul;
  renorm is deferred to the AV evacuation (1/sum is per-partition there).
- Score DR trick: the moving q operand carries an interleaved zero chunk
  and the stationary k chunk is stride-0-broadcast, so a 64-dim head
  contraction still runs at 0.5 cycles/row.
- FFN2 weight quarters are preloaded into the dead wq/wk/wv/wo SBUF slots
  during attention; FFN2 runs query-tile-major with the output DMA issued
  per tile, and its scale+residual epilogue is a single fused
  scalar_tensor_tensor.
"""

import sys

import numpy as np

sys.path.insert(0, "/opt/trn_rl_repo")

import ml_dtypes  # noqa: E402

import concourse.bass as bass  # noqa: E402
from concourse import bacc  # noqa: E402
import concourse.mybir as mybir  # noqa: E402
import concourse.tile as tile  # noqa: E402
from concourse.bass_utils import run_bass_kernel_spmd  # noqa: E402

# Pin Exp and Ln to the joint set and Gelu to its anchor set so the act-table
# load pass emits few loads instead of thrashing. Set indices are preserved.
import concourse.bacc as _bacc_mod  # noqa: E402
import concourse.hw_specs as _hw_specs  # noqa: E402

_orig_get_tables = _hw_specs.get_activation_tables


def _pinned_tables(module_arch):
    t = dict(_orig_get_tables(module_arch))
    keep = {"natural_log_exp_and_others", "gelu_and_others"}
    drop = {mybir.ActivationFunctionType.Exp,
            mybir.ActivationFunctionType.Ln,
            mybir.ActivationFunctionType.Gelu}
    return {name: (fns if name in keep else {f for f in fns if f not in drop})
            for name, fns in t.items()}


_bacc_mod.get_activation_tables = _pinned_tables


B, S, D = 2, 2048, 1024
H, DH = 16, 64
INNER = H * DH          # 1024
FF = 4 * D              # 4096
WINDOW = 128
EPS = 1e-5
SCALE = DH ** -0.5

NCORES = 8
OWN = (B * S) // NCORES          # 512 own tokens per core
HALO = WINDOW                    # 128
TLOC = OWN + 2 * HALO            # 768 local rows (halo'd)
P = 128
NQT = OWN // P                   # 4 query tiles
NKT = TLOC // P                  # 6 key tiles

F32 = mybir.dt.float32
BF16 = mybir.dt.bfloat16
FP8 = mybir.dt.float8e4
BF = ml_dtypes.bfloat16
F8NP = ml_dtypes.float8_e4m3
DR = mybir.MatmulPerfMode.DoubleRow

# fp8 weight pre-scales (cancelled on-device; see module docstring)
SQ = 8.0     # wq, wk (and their biases)
SV = 8.0     # wv
SO = 8.0     # wout
S1 = 32.0    # wff1
S2 = 64.0    # wff2
VAL = SV * SO               # validity-column value (64)
ESC = SCALE / (SQ * SQ)     # exp scale (1/512)
HV = 65                     # per-head V stride (64 dims + validity col)
MASKNEG = -1.0e6


def _build_nc(with_bias=False):
    nc = bacc.Bacc()

    x_s = nc.declare_dram_parameter("x_s", [TLOC, D], F32, isOutput=False)
    wq8 = nc.declare_dram_parameter("wq8", [P, 8, INNER], FP8, isOutput=False)
    wk8 = nc.declare_dram_parameter("wk8", [P, 8, INNER], FP8, isOutput=False)
    wv8 = nc.declare_dram_parameter("wv8", [P, 8, INNER], FP8, isOutput=False)
    wo8 = nc.declare_dram_parameter("wo8", [P, 8, D], FP8, isOutput=False)
    w18 = nc.declare_dram_parameter("w18", [4, P, 8, 1024], FP8, isOutput=False)
    w28 = nc.declare_dram_parameter("w28", [4, P, 32, 256], FP8, isOutput=False)
    bq8 = nc.declare_dram_parameter("bq8", [P, 8], F32, isOutput=False)
    bk8 = nc.declare_dram_parameter("bk8", [P, 8], F32, isOutput=False)
    bf1 = nc.declare_dram_parameter("bf1", [P, 32], F32, isOutput=False)
    brow = nc.declare_dram_parameter("brow", [1, 2 * D], BF16, isOutput=False)
    trim = nc.declare_dram_parameter("trim", [P, 2, P], BF16, isOutput=False)
    ident = nc.declare_dram_parameter("ident", [P, P], BF16, isOutput=False)
    vald = nc.declare_dram_parameter("vald", [P, NKT * H], BF16, isOutput=False)
    y = nc.declare_dram_parameter("y", [OWN, D], F32, isOutput=True)

    with tile.TileContext(nc) as tc:
        _emit(tc, nc, x_s, wq8, wk8, wv8, wo8, w18, w28,
              bq8, bk8, bf1, brow, trim, ident, vald, y, with_bias)
    nc.finalize()
    return nc


def _ln_stats(nc, small, x_ap, eps_ap):
    """Returns (mv, rstd): mv[:,0:1]=mean, rstd=[P,1] 1/sqrt(var+EPS)."""
    Ln = mybir.ActivationFunctionType.Ln
    Exp = mybir.ActivationFunctionType.Exp
    xg = x_ap.rearrange("p (s f) -> p s f", f=512)
    stats = small.tile([P, 2, 6], F32, tag="ln_stats")
    for s in range(2):
        nc.vector.bn_stats(out=stats[:, s, :], in_=xg[:, s, :])
    mv = small.tile([P, 2], F32, tag="ln_mv")
    nc.vector.bn_aggr(out=mv[:], in_=stats[:])
    rstd = small.tile([P, 1], F32, tag="ln_rstd")
    nc.scalar.activation(out=rstd[:], in_=mv[:, 1:2], func=Ln,
                         bias=eps_ap, scale=1.0)
    nc.scalar.activation(out=rstd[:], in_=rstd[:], func=Exp, scale=-0.5)
    return mv, rstd


def _emit(tc, nc, x_s, wq8, wk8, wv8, wo8, w18, w28,
          bq8, bk8, bf1, brow, trim, ident, vald, y, with_bias):
    from contextlib import ExitStack
    ctx = ExitStack()
    Gelu = mybir.ActivationFunctionType.Gelu
    Ident = mybir.ActivationFunctionType.Identity
    Copy = mybir.ActivationFunctionType.Copy
    Exp = mybir.ActivationFunctionType.Exp
    ADD = mybir.AluOpType.add
    MUL = mybir.AluOpType.mult
    SUB = mybir.AluOpType.subtract

    const = ctx.enter_context(tc.tile_pool(name="const", bufs=1))
    small = ctx.enter_context(tc.tile_pool(name="small", bufs=4))
    big = ctx.enter_context(tc.tile_pool(name="big", bufs=1))
    wst = ctx.enter_context(tc.tile_pool(name="wst", bufs=2))
    trans = ctx.enter_context(tc.tile_pool(name="trans", bufs=2))
    hot = ctx.enter_context(tc.tile_pool(name="hot", bufs=3))
    ps_big = ctx.enter_context(tc.tile_pool(name="psbig", bufs=2, space="PSUM"))
    ps_st = ctx.enter_context(tc.tile_pool(name="psst", bufs=2, space="PSUM"))
    ps_av = ctx.enter_context(tc.tile_pool(name="psav", bufs=2, space="PSUM"))

    # ---- first input tile, then V weights (PE critical path), then rest ----
    xt = big.tile([P, NKT, D], F32, tag="xt")          # 24KB/p
    nc.sync.dma_start(out=xt[:, 0, :], in_=x_s[0:P, :])
    wv_t = big.tile([P, 8, INNER], FP8, tag="wv8")     # 8KB/p
    nc.gpsimd.dma_start(out=wv_t[:, :, 0:512], in_=wv8[:, :, 0:512])
    nc.sync.dma_start(out=xt[:, 1, :], in_=x_s[P:2 * P, :])
    nc.gpsimd.dma_start(out=wv_t[:, :, 512:1024], in_=wv8[:, :, 512:1024])
    for t in range(2, NKT):
        nc.sync.dma_start(out=xt[:, t, :], in_=x_s[t * P:(t + 1) * P, :])
    wq_t = big.tile([P, 8, INNER], FP8, tag="wq8")
    nc.gpsimd.dma_start(out=wq_t[:], in_=wq8[:])
    wk_t = big.tile([P, 8, INNER], FP8, tag="wk8")
    nc.gpsimd.dma_start(out=wk_t[:], in_=wk8[:])

    # ---- constants ----
    bq_t = const.tile([P, 8], F32, tag="bq")
    nc.gpsimd.dma_start(out=bq_t[:], in_=bq8[:])
    bk_t = const.tile([P, 8], F32, tag="bk")
    nc.gpsimd.dma_start(out=bk_t[:], in_=bk8[:])
    bf1_t = const.tile([P, 32], F32, tag="bf1")
    nc.gpsimd.dma_start(out=bf1_t[:], in_=bf1[:])
    trim_t = const.tile([P, 2, P], BF16, tag="trim")
    nc.gpsimd.dma_start(out=trim_t[:], in_=trim[:])
    id_t = const.tile([P, P], BF16, tag="ident")
    nc.gpsimd.dma_start(out=id_t[:], in_=ident[:])
    if with_bias:
        brow_t = const.tile([1, 2 * D], BF16, tag="brow")
        nc.gpsimd.dma_start(out=brow_t[:], in_=brow[:])
        ones_t = const.tile([1, P], BF16, tag="ones")
        nc.vector.memset(ones_t[:], 1.0)
    eps_t = const.tile([P, 1], F32, tag="eps")
    nc.vector.memset(eps_t[:], EPS)

    # ---- LN1 -> bf16 token-major -> transpose -> fp8 feature-major ----
    yt8 = big.tile([P, 8, TLOC], FP8, tag="yt8")       # 6KB/p
    for t in range(NKT):
        mv, rstd = _ln_stats(nc, small, xt[:, t, :], eps_t[:])
        y16 = trans.tile([P, D], BF16, tag="ln16")
        nc.vector.tensor_scalar(out=y16[:], in0=xt[:, t, :],
                                scalar1=mv[:, 0:1], scalar2=rstd[:],
                                op0=SUB, op1=MUL)
        ytt = trans.tile([P, 8, P], BF16, tag="tp16")
        nc.sync.dma_start_transpose(ytt[:], y16[:])
        nc.gpsimd.tensor_copy(out=yt8[:, :, t * P:(t + 1) * P], in_=ytt[:])

    # ---- V GEMM (fp8 DR): out [tok,512] per (t, half) ----
    vtok = big.tile([P, NKT, H * HV], BF16, tag="vtok")  # ~12.2KB/p
    for t in range(NKT):
        for half in range(2):
            ps = ps_big.tile([P, 512], F32, tag="ps_big")
            for j in range(4):
                nc.tensor.matmul(ps[:], yt8[:, 2 * j:2 * j + 2, t * P:(t + 1) * P],
                                 wv_t[:, 2 * j:2 * j + 2, half * 512:(half + 1) * 512],
                                 start=(j == 0), stop=(j == 3), perf_mode=DR)
            ov = vtok[:, t, half * 8 * HV:(half + 1) * 8 * HV]
            ov = ov.rearrange("p (h d) -> p h d", d=HV)[:, :, 0:64]
            nc.vector.tensor_copy(out=ov, in_=ps[:].rearrange(
                "p (h d) -> p h d", d=64))
    # validity indicator column (also the softmax-denominator weights)
    vapd = vtok[:].rearrange("p t (h d) -> p t h d", d=HV)[:, :, :, 64]
    nc.sync.dma_start(out=vapd, in_=vald[:].rearrange(
        "p (t h) -> p t h", h=H))

    # ---- Q GEMM: out [ofeat 128, own 512] -> fp8 (with interleaved zeros) --
    qt8 = big.tile([P, 8, 2, OWN], FP8, tag="qt8")     # 8KB/p
    nc.gpsimd.memset(qt8[:, :, 1, :], 0.0)
    for o in range(8):
        ps = ps_big.tile([P, 512], F32, tag="ps_big")
        for j in range(4):
            nc.tensor.matmul(ps[:], wq_t[:, 2 * j:2 * j + 2, o * P:(o + 1) * P],
                             yt8[:, 2 * j:2 * j + 2, HALO:HALO + OWN],
                             start=(j == 0), stop=(j == 3), perf_mode=DR)
        nc.vector.tensor_scalar(out=qt8[:, o, 0, :], in0=ps[:],
                                scalar1=bq_t[:, o:o + 1], scalar2=None,
                                op0=ADD)

    # ---- K GEMM: out [ofeat 128, 2x384] -> fp8 ----
    kt8 = big.tile([P, 8, TLOC], FP8, tag="kt8")       # 6KB/p
    for o in range(8):
        ps = ps_st.tile([P, 2, 512], F32, tag="ps_st")
        for half in range(2):
            for j in range(4):
                nc.tensor.matmul(ps[:, half, 0:384],
                                 wk_t[:, 2 * j:2 * j + 2, o * P:(o + 1) * P],
                                 yt8[:, 2 * j:2 * j + 2, half * 384:(half + 1) * 384],
                                 start=(j == 0), stop=(j == 3), perf_mode=DR)
        nc.scalar.activation(out=kt8[:, o, :].rearrange("p (s c) -> p s c", c=384),
                             in_=ps[:, :, 0:384], func=Ident,
                             bias=bk_t[:, o:o + 1], scale=1.0)

    wo_t = big.tile([P, 8, D], FP8, tag="wo8")
    nc.gpsimd.dma_start(out=wo_t[:], in_=wo8[:])

    # FFN2 weight quarters -> preload into dead wq/wk/wv slots + one fresh
    w2_t = []
    for qq, tag in enumerate(["w2d", "wv8", "wq8", "wk8"]):
        w2t = big.tile([P, 32, 256], FP8, tag=tag)
        nc.gpsimd.dma_start(out=w2t[:], in_=w28[qq])
        w2_t.append(w2t)

    # ---- attention (transposed scores) + out-proj + LN2, per qtl ----
    x2 = big.tile([P, NQT, D], F32, tag="x2")          # 16KB/p
    zt8 = big.tile([P, 8, OWN], FP8, tag="zt8")        # 4KB/p
    for qtl in range(NQT):
        att = trans.tile([P, INNER], BF16, tag="att_t")
        for n in range(4):              # 4-head groups
            av = ps_av.tile([P, 4, HV], F32, tag="ps_av")
            for mm in range(2):         # 2-head subgroups
                m = 2 * n + mm
                st = ps_st.tile([P, 2, 512], F32, tag="ps_st")
                for g in range(2):
                    hs = slice(64 * g, 64 * g + 64)
                    for e in range(3):
                        ksl = slice((qtl + e) * P, (qtl + e + 1) * P)
                        lhs = kt8[hs, m, ksl].rearrange(
                            "p (o c) -> p o c", o=1).broadcast_to([64, 2, P])
                        nc.tensor.matmul(
                            st[:, g, e * P:(e + 1) * P], lhs,
                            qt8[hs, m, :, qtl * P:(qtl + 1) * P],
                            start=True, stop=(e == 1), perf_mode=DR)
                        if e != 1:      # triangle band-mask chunks
                            nc.tensor.matmul(
                                st[:, g, e * P:(e + 1) * P],
                                trim_t[:, e // 2, :], id_t[:],
                                start=False, stop=True)
                ptm = hot.tile([P, 2, 3, P], BF16, tag="ptm16")
                nc.scalar.activation(
                    out=ptm[:].rearrange("p g e q -> p g (e q)"),
                    in_=st[:, :, 0:384], func=Exp, scale=ESC)
                for g in range(2):
                    h = 2 * m + g
                    gg = 2 * mm + g
                    for e in range(3):
                        nc.tensor.matmul(
                            av[:, gg, :], ptm[:, g, e, :],
                            vtok[:, qtl + e, h * HV:(h + 1) * HV],
                            start=(e == 0), stop=(e == 2))
            rc = small.tile([P, 4], F32, tag="rc4")
            nc.vector.reciprocal(out=rc[:], in_=av[:, :, 64])
            oatt = att[:, n * 256:(n + 1) * 256].rearrange(
                "p (g d) -> p g d", d=64)
            nc.vector.tensor_tensor(out=oatt, in0=av[:, :, 0:64],
                                    in1=rc[:].broadcast_to([P, 4, 64]), op=MUL)
        atf16 = trans.tile([P, 8, P], BF16, tag="tp16")
        nc.sync.dma_start_transpose(atf16[:], att[:])
        atf8 = trans.tile([P, 8, P], FP8, tag="atf8")
        nc.vector.tensor_copy(out=atf8[:], in_=atf16[:])
        # out-projection (+bias) + residual
        for half in range(2):
            ps = ps_big.tile([P, 512], F32, tag="ps_big")
            for j in range(4):
                nc.tensor.matmul(ps[:], atf8[:, 2 * j:2 * j + 2, :],
                                 wo_t[:, 2 * j:2 * j + 2, half * 512:(half + 1) * 512],
                                 start=(j == 0), stop=(not with_bias and j == 3),
                                 perf_mode=DR)
            if with_bias:
                nc.tensor.matmul(ps[:], ones_t[:],
                                 brow_t[:, half * 512:(half + 1) * 512],
                                 start=False, stop=True)
            nc.vector.tensor_tensor(
                out=x2[:, qtl, half * 512:(half + 1) * 512], in0=ps[:],
                in1=xt[:, qtl + 1, half * 512:(half + 1) * 512], op=ADD)
        # LN2 -> bf16 -> transpose -> fp8
        mv, rstd = _ln_stats(nc, small, x2[:, qtl, :], eps_t[:])
        z16 = trans.tile([P, D], BF16, tag="ln16")
        nc.vector.tensor_scalar(out=z16[:], in0=x2[:, qtl, :],
                                scalar1=mv[:, 0:1], scalar2=rstd[:],
                                op0=SUB, op1=MUL)
        ztt = trans.tile([P, 8, P], BF16, tag="tp16")
        nc.sync.dma_start_transpose(ztt[:], z16[:])
        nc.gpsimd.tensor_copy(out=zt8[:, :, qtl * P:(qtl + 1) * P], in_=ztt[:])

    # ---- FFN1 (fp8 DR), weights streamed in 4 groups of 8 o-tiles ----
    h18 = big.tile([P, 32, OWN], FP8, tag="h18")       # 16KB/p
    for g in range(4):
        w1g = wst.tile([P, 8, 1024], FP8, tag="w1g")
        nc.gpsimd.dma_start(out=w1g[:], in_=w18[g])
        for ol in range(8):
            o = 8 * g + ol
            ps = ps_big.tile([P, 512], F32, tag="ps_big")
            for j in range(4):
                nc.tensor.matmul(ps[:], w1g[:, 2 * j:2 * j + 2, ol * P:(ol + 1) * P],
                                 zt8[:, 2 * j:2 * j + 2, :],
                                 start=(j == 0), stop=(j == 3), perf_mode=DR)
            nc.scalar.activation(out=h18[:, o, :], in_=ps[:], func=Gelu,
                                 bias=bf1_t[:, o:o + 1], scale=1.0 / S1)

    # ---- FFN2 (fp8 DR), qtl-major with fused scale+residual epilogue ----
    yo = big.tile([P, NKT, D], F32, tag="xt")          # reuses xt slot
    for qtl in range(NQT):
        for qq in range(4):
            ps = ps_big.tile([P, 512], F32, tag="ps_big")
            for j in range(16):
                nc.tensor.matmul(ps[:, 0:256],
                                 h18[:, 2 * j:2 * j + 2, qtl * P:(qtl + 1) * P],
                                 w2_t[qq][:, 2 * j:2 * j + 2, :],
                                 start=(j == 0), stop=(not with_bias and j == 15),
                                 perf_mode=DR)
            if with_bias:
                nc.tensor.matmul(ps[:, 0:256], ones_t[:],
                                 brow_t[:, D + qq * 256:D + (qq + 1) * 256],
                                 start=False, stop=True)
            nc.vector.scalar_tensor_tensor(
                out=yo[:, qtl, qq * 256:(qq + 1) * 256], in0=ps[:, 0:256],
                scalar=1.0 / S2, in1=x2[:, qtl, qq * 256:(qq + 1) * 256],
                op0=MUL, op1=ADD)
        nc.sync.dma_start(out=y[qtl * P:(qtl + 1) * P, :], in_=yo[:, qtl, :])
    ctx.close()


def _host_prep(x, ln1_g, ln1_b, w_qkv, w_out, b_out, ln2_g, ln2_b,
               w_ff1, b_ff1, w_ff2, b_ff2):
    """Fold LN affines into weights, scale + fp8-cast, build per-core maps."""
    f8 = np.float64
    wqkv_eff = (w_qkv.astype(f8) * ln1_g.astype(f8)[None, :])
    bqkv_eff = w_qkv.astype(f8) @ ln1_b.astype(f8)
    wq, wk, wv = wqkv_eff[:INNER], wqkv_eff[INNER:2 * INNER], wqkv_eff[2 * INNER:]
    bq_v, bk_v, bv_v = (bqkv_eff[:INNER], bqkv_eff[INNER:2 * INNER],
                        bqkv_eff[2 * INNER:])
    bout_eff = b_out.astype(f8) + w_out.astype(f8) @ bv_v
    wff1_eff = w_ff1.astype(f8) * ln2_g.astype(f8)[None, :]
    bff1_eff = b_ff1.astype(f8) + w_ff1.astype(f8) @ ln2_b.astype(f8)
    with_bias = not (np.all(bout_eff == 0.0) and np.all(b_ff2 == 0.0))

    def fm8(w, scale):
        # w [N, Dk] -> [128, Dk//128, N] fp8: [p, kc, n] = w[n, 128*kc+p]*scale
        dk = w.shape[1]
        a = (w.T * scale).reshape(dk // P, P, -1).transpose(1, 0, 2)
        return np.ascontiguousarray(a).astype(F8NP)

    wq8 = fm8(wq, SQ)
    wk8 = fm8(wk, SQ)
    wv8 = fm8(wv, SV)
    wo8 = fm8(w_out.astype(f8), SO)
    w18_full = fm8(wff1_eff, S1)                     # [128, 8, 4096]
    w18 = np.ascontiguousarray(
        w18_full.reshape(P, 8, 4, 1024).transpose(2, 0, 1, 3))
    w28_full = fm8(w_ff2.astype(f8), S2)             # [128, 32, 1024]
    w28 = np.ascontiguousarray(
        w28_full.reshape(P, 32, 4, 256).transpose(2, 0, 1, 3))

    def colmaj(b, n, scale=1.0):
        return np.ascontiguousarray(
            (b * scale).reshape(n, P).T).astype(np.float32)

    bq8 = colmaj(bq_v, 8, SQ)
    bk8 = colmaj(bk_v, 8, SQ)
    bf1 = colmaj(bff1_eff, 32)
    brow = np.concatenate([bout_eff, b_ff2.astype(f8) * S2]).reshape(1, 2 * D)
    brow = brow.astype(BF)

    # triangle additive masks, stored transposed for lhsT.T @ I injection:
    # trim[0][r, pk] covers e=0 (allowed pk >= r); trim[1] e=2 (pk <= r)
    pk = np.arange(P)[None, :]
    qq_i = np.arange(P)[:, None]
    trim = np.stack([
        np.where(pk >= qq_i, 0.0, MASKNEG),    # e=0: allowed pk >= q
        np.where(pk <= qq_i, 0.0, MASKNEG),    # e=2: allowed pk <= q
    ], axis=1).astype(BF)                       # [q, eidx, pk]
    identm = np.eye(P, dtype=BF)

    xf = x.reshape(B * S, D).astype(np.float32)
    in_maps = []
    for c in range(NCORES):
        b = c // (NCORES // B)
        s0 = (c % (NCORES // B)) * OWN
        lo, hi = s0 - HALO, s0 + OWN + HALO
        xs = np.zeros((TLOC, D), np.float32)
        clo, chi = max(lo, 0), min(hi, S)
        xs[clo - lo:chi - lo] = xf[b * S + clo:b * S + chi]
        lt = np.arange(TLOC)
        valid = ((s0 - HALO + lt) >= 0) & ((s0 - HALO + lt) < S)
        vald = np.where(valid[:, None], np.float32(VAL), np.float32(0.0))
        vald = np.broadcast_to(vald, (TLOC, H)).reshape(NKT, P, H)
        vald = np.ascontiguousarray(vald.transpose(1, 0, 2)).reshape(
            P, NKT * H).astype(BF)
        in_maps.append(dict(
            x_s=xs, wq8=wq8, wk8=wk8, wv8=wv8, wo8=wo8, w18=w18, w28=w28,
            bq8=bq8, bk8=bk8, bf1=bf1, brow=brow, trim=trim, ident=identm,
            vald=vald))
    return in_maps, with_bias


_NC_CACHE = {}


def kernel(x, ln1_g, ln1_b, w_qkv, w_out, b_out, ln2_g, ln2_b,
           w_ff1, b_ff1, w_ff2, b_ff2, _trace=False):
    in_maps, with_bias = _host_prep(x, ln1_g, ln1_b, w_qkv, w_out, b_out,
                                    ln2_g, ln2_b, w_ff1, b_ff1, w_ff2, b_ff2)
    key = ("nc", with_bias)
    if key not in _NC_CACHE:
        _NC_CACHE[key] = _build_nc(with_bias)
    nc = _NC_CACHE[key]
    res = run_bass_kernel_spmd(nc, in_maps, core_ids=list(range(NCORES)),
                               trace=_trace)
    if _trace:
        _NC_CACHE["last"] = res
    out = np.concatenate([res.results[c]["y"] for c in range(NCORES)], axis=0)
    return out.reshape(B, S, D).astype(np.float32)


# revision 5
# speedup vs baseline: 2.3969x; 1.1232x over previous
"""Trainium2 Bass kernel for nn_ASAMLayer (local-window sparse attention layer).

v3: fp8 DoubleRow everywhere (incl. scores) + matmul-injected band mask.

Sharding: token-parallel across 8 cores (512 own tokens + 128-halo each side).

Structure per core:
- LN1 on token-major f32 x -> bf16 -> DmaTranspose -> fp8 feature-major x_hat.
- All big GEMMs (V/Q/K/scores/out-proj/FFN1/FFN2) are fp8e4m3 DoubleRow
  matmuls (2 contraction chunks per instruction at 0.5 cycles/row).  Weights
  are pre-scaled on the host (x8/x32/x64) to escape fp8's denormal range;
  the scales cancel in the exp scale, the gelu input scale, a fused 1/64
  output scale, and the softmax-denominator validity column.
- Attention computed transposed: scores land as [key, query] tiles so the
  softmax probabilities feed the AV matmul directly as the stationary
  operand (no per-head transposes).  The |i-j|<=128 band mask decomposes
  per 128-key chunk into [lower-tri, all-ones, upper-tri]; the triangle
  chunks get a -1e6 additive mask injected into PSUM by a bf16 matmul
  (triM^T @ I) before the exp, so no vector-engine mask pass exists.
  Sequence-edge validity is a scaled indicator column appended to V which
  simultaneously produces the softmax denominators inside the AV matmul;
  renorm is deferred to the AV evacuation (1/sum is per-partition there).
- Score DR trick: the moving q operand carries an interleaved zero chunk
  and the stationary k chunk is stride-0-broadcast, so a 64-dim head
  contraction still runs at 0.5 cycles/row.
- FFN2 weight quarters are preloaded into the dead wq/wk/wv/wo SBUF slots
  during attention; FFN2 runs query-tile-major with the output DMA issued
  per tile, and its scale+residual epilogue is a single fused
  scalar_tensor_tensor.
"""

import sys

import numpy as np

sys.path.insert(0, "/opt/trn_rl_repo")

import ml_dtypes  # noqa: E402

import concourse.bass as bass  # noqa: E402
from concourse import bacc  # noqa: E402
import concourse.mybir as mybir  # noqa: E402
import concourse.tile as tile  # noqa: E402
from concourse.bass_utils import run_bass_kernel_spmd  # noqa: E402

# Pin Exp and Ln to the joint set and Gelu to its anchor set so the act-table
# load pass emits few loads instead of thrashing. Set indices are preserved.
import concourse.bacc as _bacc_mod  # noqa: E402
import concourse.hw_specs as _hw_specs  # noqa: E402

_orig_get_tables = _hw_specs.get_activation_tables


def _pinned_tables(module_arch):
    t = dict(_orig_get_tables(module_arch))
    keep = {"natural_log_exp_and_others", "gelu_and_others"}
    drop = {mybir.ActivationFunctionType.Exp,
            mybir.ActivationFunctionType.Ln,
            mybir.ActivationFunctionType.Gelu}
    return {name: (fns if name in keep else {f for f in fns if f not in drop})
            for name, fns in t.items()}


_bacc_mod.get_activation_tables = _pinned_tables


B, S, D = 2, 2048, 1024
H, DH = 16, 64
INNER = H * DH          # 1024
FF = 4 * D              # 4096
WINDOW = 128
EPS = 1e-5
SCALE = DH ** -0.5

NCORES = 8
OWN = (B * S) // NCORES          # 512 own tokens per core
HALO = WINDOW                    # 128
TLOC = OWN + 2 * HALO            # 768 local rows (halo'd)
P = 128
NQT = OWN // P                   # 4 query tiles
NKT = TLOC // P                  # 6 key tiles

F32 = mybir.dt.float32
BF16 = mybir.dt.bfloat16
FP8 = mybir.dt.float8e4
BF = ml_dtypes.bfloat16
F8NP = ml_dtypes.float8_e4m3
DR = mybir.MatmulPerfMode.DoubleRow

# fp8 weight pre-scales (cancelled on-device; see module docstring)
SQ = 8.0     # wq, wk (and their biases)
SV = 8.0     # wv
SO = 8.0     # wout
S1 = 32.0    # wff1
S2 = 64.0    # wff2
VAL = SV * SO               # validity-column value (64)
ESC = SCALE / (SQ * SQ)     # exp scale (1/512)
HV = 65                     # per-head V stride (64 dims + validity col)
MASKNEG = -1.0e6


def _build_nc(with_bias=False):
    nc = bacc.Bacc()

    x_s = nc.declare_dram_parameter("x_s", [TLOC, D], BF16, isOutput=False)
    wq8 = nc.declare_dram_parameter("wq8", [P, 8, INNER], FP8, isOutput=False)
    wk8 = nc.declare_dram_parameter("wk8", [P, 8, INNER], FP8, isOutput=False)
    wv8 = nc.declare_dram_parameter("wv8", [P, 8, INNER], FP8, isOutput=False)
    wo8 = nc.declare_dram_parameter("wo8", [P, 8, D], FP8, isOutput=False)
    w18 = nc.declare_dram_parameter("w18", [4, P, 8, 1024], FP8, isOutput=False)
    w28 = nc.declare_dram_parameter("w28", [4, P, 32, 256], FP8, isOutput=False)
    bq8 = nc.declare_dram_parameter("bq8", [P, 8], F32, isOutput=False)
    bk8 = nc.declare_dram_parameter("bk8", [P, 8], F32, isOutput=False)
    bf1 = nc.declare_dram_parameter("bf1", [P, 32], F32, isOutput=False)
    brow = nc.declare_dram_parameter("brow", [1, 2 * D], BF16, isOutput=False)
    trim = nc.declare_dram_parameter("trim", [P, 2, P], BF16, isOutput=False)
    ident = nc.declare_dram_parameter("ident", [P, P], BF16, isOutput=False)
    vald = nc.declare_dram_parameter("vald", [P, NKT * H], BF16, isOutput=False)
    y = nc.declare_dram_parameter("y", [OWN, D], F32, isOutput=True)

    with tile.TileContext(nc) as tc:
        _emit(tc, nc, x_s, wq8, wk8, wv8, wo8, w18, w28,
              bq8, bk8, bf1, brow, trim, ident, vald, y, with_bias)
    nc.finalize()
    return nc


def _ln_stats(nc, small, x_ap, eps_ap):
    """Returns (mv, rstd): mv[:,0:1]=mean, rstd=[P,1] 1/sqrt(var+EPS)."""
    Ln = mybir.ActivationFunctionType.Ln
    Exp = mybir.ActivationFunctionType.Exp
    xg = x_ap.rearrange("p (s f) -> p s f", f=512)
    stats = small.tile([P, 2, 6], F32, tag="ln_stats")
    for s in range(2):
        nc.vector.bn_stats(out=stats[:, s, :], in_=xg[:, s, :])
    mv = small.tile([P, 2], F32, tag="ln_mv")
    nc.vector.bn_aggr(out=mv[:], in_=stats[:])
    rstd = small.tile([P, 1], F32, tag="ln_rstd")
    nc.scalar.activation(out=rstd[:], in_=mv[:, 1:2], func=Ln,
                         bias=eps_ap, scale=1.0)
    nc.scalar.activation(out=rstd[:], in_=rstd[:], func=Exp, scale=-0.5)
    return mv, rstd


def _emit(tc, nc, x_s, wq8, wk8, wv8, wo8, w18, w28,
          bq8, bk8, bf1, brow, trim, ident, vald, y, with_bias):
    from contextlib import ExitStack
    ctx = ExitStack()
    Gelu = mybir.ActivationFunctionType.Gelu
    Ident = mybir.ActivationFunctionType.Identity
    Copy = mybir.ActivationFunctionType.Copy
    Exp = mybir.ActivationFunctionType.Exp
    ADD = mybir.AluOpType.add
    MUL = mybir.AluOpType.mult
    SUB = mybir.AluOpType.subtract

    const = ctx.enter_context(tc.tile_pool(name="const", bufs=1))
    small = ctx.enter_context(tc.tile_pool(name="small", bufs=4))
    big = ctx.enter_context(tc.tile_pool(name="big", bufs=1))
    wst = ctx.enter_context(tc.tile_pool(name="wst", bufs=2))
    trans = ctx.enter_context(tc.tile_pool(name="trans", bufs=2))
    hot = ctx.enter_context(tc.tile_pool(name="hot", bufs=3))
    ps_big = ctx.enter_context(tc.tile_pool(name="psbig", bufs=2, space="PSUM"))
    ps_st = ctx.enter_context(tc.tile_pool(name="psst", bufs=2, space="PSUM"))
    ps_av = ctx.enter_context(tc.tile_pool(name="psav", bufs=2, space="PSUM"))

    # ---- first input tile, then V weights (PE critical path), then rest ----
    xt = big.tile([P, NKT, D], BF16, tag="xt")         # 12KB/p
    nc.sync.dma_start(out=xt[:, 0, :], in_=x_s[0:P, :])
    wv_t = big.tile([P, 8, INNER], FP8, tag="wv8")     # 8KB/p
    nc.gpsimd.dma_start(out=wv_t[:, :, 0:512], in_=wv8[:, :, 0:512])
    nc.sync.dma_start(out=xt[:, 1, :], in_=x_s[P:2 * P, :])
    nc.gpsimd.dma_start(out=wv_t[:, :, 512:1024], in_=wv8[:, :, 512:1024])
    for t in range(2, NKT):
        nc.sync.dma_start(out=xt[:, t, :], in_=x_s[t * P:(t + 1) * P, :])
    wq_t = big.tile([P, 8, INNER], FP8, tag="wq8")
    nc.gpsimd.dma_start(out=wq_t[:], in_=wq8[:])
    wk_t = big.tile([P, 8, INNER], FP8, tag="wk8")
    nc.gpsimd.dma_start(out=wk_t[:], in_=wk8[:])

    # ---- constants ----
    bq_t = const.tile([P, 8], F32, tag="bq")
    nc.sync.dma_start(out=bq_t[:], in_=bq8[:])
    bk_t = const.tile([P, 8], F32, tag="bk")
    nc.sync.dma_start(out=bk_t[:], in_=bk8[:])
    bf1_t = const.tile([P, 32], F32, tag="bf1")
    nc.sync.dma_start(out=bf1_t[:], in_=bf1[:])
    trim_t = const.tile([P, 2, P], BF16, tag="trim")
    nc.sync.dma_start(out=trim_t[:], in_=trim[:])
    id_t = const.tile([P, P], BF16, tag="ident")
    nc.sync.dma_start(out=id_t[:], in_=ident[:])
    if with_bias:
        brow_t = const.tile([1, 2 * D], BF16, tag="brow")
        nc.sync.dma_start(out=brow_t[:], in_=brow[:])
        ones_t = const.tile([1, P], BF16, tag="ones")
        nc.vector.memset(ones_t[:], 1.0)
    eps_t = const.tile([P, 1], F32, tag="eps")
    nc.vector.memset(eps_t[:], EPS)

    # ---- LN1 -> bf16 token-major -> transpose -> fp8 feature-major ----
    yt8 = big.tile([P, 8, TLOC], FP8, tag="yt8")       # 6KB/p
    for t in range(NKT):
        mv, rstd = _ln_stats(nc, small, xt[:, t, :], eps_t[:])
        y16 = trans.tile([P, D], BF16, tag="ln16")
        nc.vector.tensor_scalar(out=y16[:], in0=xt[:, t, :],
                                scalar1=mv[:, 0:1], scalar2=rstd[:],
                                op0=SUB, op1=MUL)
        ytt = trans.tile([P, 8, P], BF16, tag="tp16")
        nc.sync.dma_start_transpose(ytt[:], y16[:])
        nc.scalar.activation(out=yt8[:, :, t * P:(t + 1) * P], in_=ytt[:],
                             func=Copy, scale=1.0)

    # ---- V GEMM (fp8 DR): out [tok,512] per (t, half) ----
    vtok = big.tile([P, NKT, H * HV], BF16, tag="vtok")  # ~12.2KB/p
    for t in range(NKT):
        ps = ps_st.tile([P, 2, 512], F32, tag="ps_st")
        for half in range(2):
            for j in range(4):
                nc.tensor.matmul(ps[:, half, :], yt8[:, 2 * j:2 * j + 2, t * P:(t + 1) * P],
                                 wv_t[:, 2 * j:2 * j + 2, half * 512:(half + 1) * 512],
                                 start=(j == 0), stop=(j == 3), perf_mode=DR)
        ov = vtok[:, t, :].rearrange("p (hf h d) -> p hf h d", hf=2, d=HV)[:, :, :, 0:64]
        nc.vector.tensor_copy(out=ov, in_=ps[:].rearrange(
            "p hf (h d) -> p hf h d", d=64))
    # validity indicator column (also the softmax-denominator weights)
    vapd = vtok[:].rearrange("p t (h d) -> p t h d", d=HV)[:, :, :, 64]
    nc.sync.dma_start(out=vapd, in_=vald[:].rearrange(
        "p (t h) -> p t h", h=H))

    # ---- Q GEMM: out [ofeat 128, own 512] -> fp8 (with interleaved zeros) --
    qt8 = big.tile([P, 2, 8, OWN], FP8, tag="qt8")     # 8KB/p
    nc.gpsimd.memset(qt8[:, 1, :, :], 0.0)
    for o in range(8):
        ps = ps_big.tile([P, 512], F32, tag="ps_big")
        for j in range(4):
            nc.tensor.matmul(ps[:], wq_t[:, 2 * j:2 * j + 2, o * P:(o + 1) * P],
                             yt8[:, 2 * j:2 * j + 2, HALO:HALO + OWN],
                             start=(j == 0), stop=(j == 3), perf_mode=DR)
        if with_bias:
            nc.vector.tensor_scalar(out=qt8[:, 0, o, :], in0=ps[:],
                                    scalar1=bq_t[:, o:o + 1], scalar2=None,
                                    op0=ADD)
        else:
            nc.vector.tensor_copy(out=qt8[:, 0, o, :], in_=ps[:])

    # ---- K GEMM: out [ofeat 128, 2x384] -> fp8 ----
    kt8 = big.tile([P, 8, TLOC], FP8, tag="kt8")       # 6KB/p
    for o in range(8):
        ps = ps_st.tile([P, 2, 512], F32, tag="ps_st")
        for half in range(2):
            for j in range(4):
                nc.tensor.matmul(ps[:, half, 0:384],
                                 wk_t[:, 2 * j:2 * j + 2, o * P:(o + 1) * P],
                                 yt8[:, 2 * j:2 * j + 2, half * 384:(half + 1) * 384],
                                 start=(j == 0), stop=(j == 3), perf_mode=DR)
        nc.scalar.activation(out=kt8[:, o, :].rearrange("p (s c) -> p s c", c=384),
                             in_=ps[:, :, 0:384], func=Ident,
                             bias=(bk_t[:, o:o + 1] if with_bias else 0.0),
                             scale=1.0)

    wo_t = big.tile([P, 8, D], FP8, tag="wo8")
    nc.gpsimd.dma_start(out=wo_t[:], in_=wo8[:])

    # FFN2 weight quarters -> preload into dead wq/wk/wv slots + one fresh
    w2_t = []
    for qq, tag in enumerate(["w2d", "wv8", "wq8", "wk8"]):
        w2t = big.tile([P, 32, 256], FP8, tag=tag)
        nc.gpsimd.dma_start(out=w2t[:], in_=w28[qq])
        w2_t.append(w2t)

    # ---- attention (transposed scores) + out-proj + LN2, per qtl ----
    x2 = big.tile([P, NQT, D], F32, tag="x2")          # 16KB/p
    zt8 = big.tile([P, 8, OWN], FP8, tag="zt8")        # 4KB/p
    for qtl in range(NQT):
        att = trans.tile([P, INNER], BF16, tag="att_t")
        for n in range(4):              # 4-head groups
            av = ps_av.tile([P, 4, HV], F32, tag="ps_av")
            for mm in range(2):         # 2-head subgroups
                m = 2 * n + mm
                st = ps_st.tile([P, 2, 512], F32, tag="ps_st")
                for g in range(2):
                    hs = slice(64 * g, 64 * g + 64)
                    for e in range(3):
                        ksl = slice((qtl + e) * P, (qtl + e + 1) * P)
                        lhs = kt8[hs, m, ksl].rearrange(
                            "p (o c) -> p o c", o=1).broadcast_to([64, 2, P])
                        nc.tensor.matmul(
                            st[:, g, e * P:(e + 1) * P], lhs,
                            qt8[hs, :, m, qtl * P:(qtl + 1) * P],
                            start=True, stop=(e == 1), perf_mode=DR)
                        if e != 1:      # triangle band-mask chunks
                            nc.tensor.matmul(
                                st[:, g, e * P:(e + 1) * P],
                                trim_t[:, e // 2, :], id_t[:],
                                start=False, stop=True)
                ptm = hot.tile([P, 2, 3, P], BF16, tag="ptm16")
                nc.scalar.activation(
                    out=ptm[:].rearrange("p g e q -> p g (e q)"),
                    in_=st[:, :, 0:384], func=Exp, scale=ESC)
                for g in range(2):
                    h = 2 * m + g
                    gg = 2 * mm + g
                    for e in range(3):
                        nc.tensor.matmul(
                            av[:, gg, :], ptm[:, g, e, :],
                            vtok[:, qtl + e, h * HV:(h + 1) * HV],
                            start=(e == 0), stop=(e == 2))
            rc = small.tile([P, 4], F32, tag="rc4")
            nc.vector.reciprocal(out=rc[:], in_=av[:, :, 64])
            oatt = att[:, n * 256:(n + 1) * 256].rearrange(
                "p (g d) -> p g d", d=64)
            nc.vector.tensor_tensor(out=oatt, in0=av[:, :, 0:64],
                                    in1=rc[:].broadcast_to([P, 4, 64]), op=MUL)
        atf16 = trans.tile([P, 8, P], BF16, tag="tp16")
        nc.sync.dma_start_transpose(atf16[:], att[:])
        atf8 = trans.tile([P, 8, P], FP8, tag="atf8")
        nc.vector.tensor_copy(out=atf8[:], in_=atf16[:])
        # out-projection (+bias) + residual
        for half in range(2):
            ps = ps_big.tile([P, 512], F32, tag="ps_big")
            for j in range(4):
                nc.tensor.matmul(ps[:], atf8[:, 2 * j:2 * j + 2, :],
                                 wo_t[:, 2 * j:2 * j + 2, half * 512:(half + 1) * 512],
                                 start=(j == 0), stop=(not with_bias and j == 3),
                                 perf_mode=DR)
            if with_bias:
                nc.tensor.matmul(ps[:], ones_t[:],
                                 brow_t[:, half * 512:(half + 1) * 512],
                                 start=False, stop=True)
            nc.vector.tensor_tensor(
                out=x2[:, qtl, half * 512:(half + 1) * 512], in0=ps[:],
                in1=xt[:, qtl + 1, half * 512:(half + 1) * 512], op=ADD)
        # LN2 -> bf16 -> transpose -> fp8
        mv, rstd = _ln_stats(nc, small, x2[:, qtl, :], eps_t[:])
        z16 = trans.tile([P, D], BF16, tag="ln16")
        nc.vector.tensor_scalar(out=z16[:], in0=x2[:, qtl, :],
                                scalar1=mv[:, 0:1], scalar2=rstd[:],
                                op0=SUB, op1=MUL)
        ztt = trans.tile([P, 8, P], BF16, tag="tp16")
        nc.sync.dma_start_transpose(ztt[:], z16[:])
        nc.scalar.activation(out=zt8[:, :, qtl * P:(qtl + 1) * P], in_=ztt[:],
                             func=Copy, scale=1.0)

    # ---- FFN1 (fp8 DR), weights streamed in 4 groups of 8 o-tiles ----
    h18 = big.tile([P, 32, OWN], FP8, tag="h18")       # 16KB/p
    for g in range(4):
        w1g = wst.tile([P, 8, 1024], FP8, tag="w1g")
        nc.gpsimd.dma_start(out=w1g[:], in_=w18[g])
        if not with_bias:
            for op_ in range(4):
                ps = ps_st.tile([P, 2, 512], F32, tag="ps_st")
                for oi in range(2):
                    ol = 2 * op_ + oi
                    for j in range(4):
                        nc.tensor.matmul(ps[:, oi, :],
                                         w1g[:, 2 * j:2 * j + 2, ol * P:(ol + 1) * P],
                                         zt8[:, 2 * j:2 * j + 2, :],
                                         start=(j == 0), stop=(j == 3), perf_mode=DR)
                o0 = 8 * g + 2 * op_
                nc.scalar.activation(out=h18[:, o0:o0 + 2, :], in_=ps[:],
                                     func=Gelu, scale=1.0 / S1)
        else:
            for ol in range(8):
                o = 8 * g + ol
                ps = ps_big.tile([P, 512], F32, tag="ps_big")
                for j in range(4):
                    nc.tensor.matmul(ps[:], w1g[:, 2 * j:2 * j + 2, ol * P:(ol + 1) * P],
                                     zt8[:, 2 * j:2 * j + 2, :],
                                     start=(j == 0), stop=(j == 3), perf_mode=DR)
                nc.scalar.activation(out=h18[:, o, :], in_=ps[:], func=Gelu,
                                     bias=bf1_t[:, o:o + 1], scale=1.0 / S1)

    # ---- FFN2 (fp8 DR), qtl-major with fused scale+residual epilogue ----
    yo = big.tile([P, NQT, D], F32, tag="yo")          # 16KB/p
    for qtl in range(NQT):
        for qq in range(4):
            ps = ps_big.tile([P, 512], F32, tag="ps_big")
            for j in range(16):
                nc.tensor.matmul(ps[:, 0:256],
                                 h18[:, 2 * j:2 * j + 2, qtl * P:(qtl + 1) * P],
                                 w2_t[qq][:, 2 * j:2 * j + 2, :],
                                 start=(j == 0), stop=(not with_bias and j == 15),
                                 perf_mode=DR)
            if with_bias:
                nc.tensor.matmul(ps[:, 0:256], ones_t[:],
                                 brow_t[:, D + qq * 256:D + (qq + 1) * 256],
                                 start=False, stop=True)
            nc.vector.scalar_tensor_tensor(
                out=yo[:, qtl, qq * 256:(qq + 1) * 256], in0=ps[:, 0:256],
                scalar=1.0 / S2, in1=x2[:, qtl, qq * 256:(qq + 1) * 256],
                op0=MUL, op1=ADD)
        nc.sync.dma_start(out=y[qtl * P:(qtl + 1) * P, :], in_=yo[:, qtl, :])
    ctx.close()


def _host_prep(x, ln1_g, ln1_b, w_qkv, w_out, b_out, ln2_g, ln2_b,
               w_ff1, b_ff1, w_ff2, b_ff2):
    """Fold LN affines into weights, scale + fp8-cast, build per-core maps."""
    f8 = np.float64
    wqkv_eff = (w_qkv.astype(f8) * ln1_g.astype(f8)[None, :])
    bqkv_eff = w_qkv.astype(f8) @ ln1_b.astype(f8)
    wq, wk, wv = wqkv_eff[:INNER], wqkv_eff[INNER:2 * INNER], wqkv_eff[2 * INNER:]
    bq_v, bk_v, bv_v = (bqkv_eff[:INNER], bqkv_eff[INNER:2 * INNER],
                        bqkv_eff[2 * INNER:])
    bout_eff = b_out.astype(f8) + w_out.astype(f8) @ bv_v
    wff1_eff = w_ff1.astype(f8) * ln2_g.astype(f8)[None, :]
    bff1_eff = b_ff1.astype(f8) + w_ff1.astype(f8) @ ln2_b.astype(f8)
    with_bias = not (np.all(bout_eff == 0.0) and np.all(b_ff2 == 0.0)
                     and np.all(bqkv_eff == 0.0) and np.all(bff1_eff == 0.0))

    def fm8(w, scale):
        # w [N, Dk] -> [128, Dk//128, N] fp8: [p, kc, n] = w[n, 128*kc+p]*scale
        dk = w.shape[1]
        a = (w.T * scale).reshape(dk // P, P, -1).transpose(1, 0, 2)
        return np.ascontiguousarray(a).astype(F8NP)

    wq8 = fm8(wq, SQ)
    wk8 = fm8(wk, SQ)
    wv8 = fm8(wv, SV)
    wo8 = fm8(w_out.astype(f8), SO)
    w18_full = fm8(wff1_eff, S1)                     # [128, 8, 4096]
    w18 = np.ascontiguousarray(
        w18_full.reshape(P, 8, 4, 1024).transpose(2, 0, 1, 3))
    w28_full = fm8(w_ff2.astype(f8), S2)             # [128, 32, 1024]
    w28 = np.ascontiguousarray(
        w28_full.reshape(P, 32, 4, 256).transpose(2, 0, 1, 3))

    def colmaj(b, n, scale=1.0):
        return np.ascontiguousarray(
            (b * scale).reshape(n, P).T).astype(np.float32)

    bq8 = colmaj(bq_v, 8, SQ)
    bk8 = colmaj(bk_v, 8, SQ)
    bf1 = colmaj(bff1_eff, 32)
    brow = np.concatenate([bout_eff, b_ff2.astype(f8) * S2]).reshape(1, 2 * D)
    brow = brow.astype(BF)

    # triangle additive masks, stored transposed for lhsT.T @ I injection:
    # trim[0][r, pk] covers e=0 (allowed pk >= r); trim[1] e=2 (pk <= r)
    pk = np.arange(P)[None, :]
    qq_i = np.arange(P)[:, None]
    trim = np.stack([
        np.where(pk >= qq_i, 0.0, MASKNEG),    # e=0: allowed pk >= q
        np.where(pk <= qq_i, 0.0, MASKNEG),    # e=2: allowed pk <= q
    ], axis=1).astype(BF)                       # [q, eidx, pk]
    identm = np.eye(P, dtype=BF)

    xf = x.reshape(B * S, D).astype(np.float32)
    in_maps = []
    for c in range(NCORES):
        b = c // (NCORES // B)
        s0 = (c % (NCORES // B)) * OWN
        lo, hi = s0 - HALO, s0 + OWN + HALO
        xs = np.zeros((TLOC, D), BF)
        clo, chi = max(lo, 0), min(hi, S)
        xs[clo - lo:chi - lo] = xf[b * S + clo:b * S + chi]
        lt = np.arange(TLOC)
        valid = ((s0 - HALO + lt) >= 0) & ((s0 - HALO + lt) < S)
        vald = np.where(valid[:, None], np.float32(VAL), np.float32(0.0))
        vald = np.broadcast_to(vald, (TLOC, H)).reshape(NKT, P, H)
        vald = np.ascontiguousarray(vald.transpose(1, 0, 2)).reshape(
            P, NKT * H).astype(BF)
        in_maps.append(dict(
            x_s=xs, wq8=wq8, wk8=wk8, wv8=wv8, wo8=wo8, w18=w18, w28=w28,
            bq8=bq8, bk8=bk8, bf1=bf1, brow=brow, trim=trim, ident=identm,
            vald=vald))
    return in_maps, with_bias


_NC_CACHE = {}


def kernel(x, ln1_g, ln1_b, w_qkv, w_out, b_out, ln2_g, ln2_b,
           w_ff1, b_ff1, w_ff2, b_ff2, _trace=False):
    in_maps, with_bias = _host_prep(x, ln1_g, ln1_b, w_qkv, w_out, b_out,
                                    ln2_g, ln2_b, w_ff1, b_ff1, w_ff2, b_ff2)
    key = ("nc", with_bias)
    if key not in _NC_CACHE:
        _NC_CACHE[key] = _build_nc(with_bias)
    nc = _NC_CACHE[key]
    res = run_bass_kernel_spmd(nc, in_maps, core_ids=list(range(NCORES)),
                               trace=_trace)
    if _trace:
        _NC_CACHE["last"] = res
    out = np.concatenate([res.results[c]["y"] for c in range(NCORES)], axis=0)
    return out.reshape(B, S, D).astype(np.float32)
